# revision 52
# baseline (speedup 1.0000x reference)
"""Windowed attention block (LeViT/Swin-style) on Trainium2, 8 NeuronCores.

LayerNorm -> QKV -> per-head biased softmax attention -> projection for
B=256 windows, N=196 tokens, DIM=384, 12 heads of head-dim 32.

Sharding: data-parallel over windows (32 windows/core, 8 cores); params
replicated. Per core the Bass/Tile kernel processes windows one at a time:

  stage A: x tile [t<=128, 384] -> LN stats (bn_stats) -> xn bf16
           -> PE-transpose -> xnT [384, 196] (feature-major)
  stage B: QKV. q/k feature-major ([384 f, 196 t], head-grouped rows,
           scale folded into wq), v token-major ([196 t, 384 f]).
  stage C: per head: scores = qT.T @ kT -> PSUM [n, m]; softmax along
           free dim (exp on ACT, x exp(bias), rowsum, reciprocal,
           normalize with a zero-stride broadcast multiply); PE-transpose
           probs -> probsT [m, n]; AV: out = v.T @ probsT -> oT [f, n];
           proj (fp32r) -> [t, 384] -> DMA out.

All matmuls bf16 except the projection (fp32r). LN gamma/beta are folded
into the QKV weights on the host; softmax max-subtraction is skipped
(scores are O(1) by construction: weights ~N(0, 0.02^2)).
"""

import sys

sys.path.insert(0, "/opt/trn_rl_repo")

import numpy as np
import ml_dtypes

B, N, DIM = 256, 196, 384
H, KD, VD = 12, 32, 32
EPS = 1e-5
NCORES = 8
WPC = B // NCORES  # windows per core

BF16 = ml_dtypes.bfloat16

_cache = {}
last_results = None


STOP_AFTER = None  # bisect hook: "A","B","scores","softmax","transp","av",None


def _build(n_windows, has_bias, debug=False):
    stop = STOP_AFTER
    import concourse.bass as bass
    import concourse.bacc as bacc
    import concourse.tile as tile
    from concourse import mybir
    from concourse.masks import make_identity

    f32 = mybir.dt.float32
    f32r = mybir.dt.float32r
    bf16 = mybir.dt.bfloat16
    AF = mybir.ActivationFunctionType
    ALU = mybir.AluOpType

    nc = bacc.Bacc("TRN2", target_bir_lowering=False, debug=False)

    x_d = nc.dram_tensor("x", [n_windows, N, DIM], f32, kind="ExternalInput")
    wq_d = nc.dram_tensor("wqT", [128, 3, 384], bf16, kind="ExternalInput")
    wk_d = nc.dram_tensor("wkT", [128, 3, 384], bf16, kind="ExternalInput")
    wv_d = nc.dram_tensor("wvT", [128, 3, 384], bf16, kind="ExternalInput")
    pw_d = nc.dram_tensor("pwT", [128, 3, 384], f32r, kind="ExternalInput")
    eb_d = nc.dram_tensor("expb", [N, H, N], bf16, kind="ExternalInput")
    ind_d = nc.dram_tensor("ind", [H, 3, 128], bf16, kind="ExternalInput")
    if has_bias:
        bqk_d = nc.dram_tensor("bqk", [128, 6], f32, kind="ExternalInput")
        bvp_d = nc.dram_tensor("bvp", [2, 384], f32, kind="ExternalInput")
    out_d = nc.dram_tensor("out", [n_windows, N, DIM], f32, kind="ExternalOutput")
    if debug:
        dbg = {
            "d_xnT": nc.dram_tensor("d_xnT", [128, 3, N], bf16, kind="ExternalOutput"),
            "d_qT": nc.dram_tensor("d_qT", [128, 3, N], bf16, kind="ExternalOutput"),
            "d_kT": nc.dram_tensor("d_kT", [128, 3, N], bf16, kind="ExternalOutput"),
            "d_vw": nc.dram_tensor("d_vw", [128, 2, 384], bf16, kind="ExternalOutput"),
            "d_P0": nc.dram_tensor("d_P0", [128, H, N], bf16, kind="ExternalOutput"),
            "d_pT0": nc.dram_tensor("d_pT0", [128, H, N], bf16, kind="ExternalOutput"),
            "d_oT": nc.dram_tensor("d_oT", [128, 3, N], f32r, kind="ExternalOutput"),
        }

    xa, outa = x_d.ap(), out_d.ap()

    # token chunks of a window: [0,128) and [128,196)
    CH = [(0, 128), (128, N)]

    with tile.TileContext(nc) as tc:
        import contextlib

        ctx = contextlib.ExitStack()
        with ctx:
            consts = ctx.enter_context(tc.tile_pool(name="consts", bufs=1))
            x_pool = ctx.enter_context(tc.tile_pool(name="x", bufs=3))
            ln_pool = ctx.enter_context(tc.tile_pool(name="ln", bufs=4))
            xn_pool = ctx.enter_context(tc.tile_pool(name="xn", bufs=3))
            xnt_pool = ctx.enter_context(tc.tile_pool(name="xnt", bufs=3))
            qk_pool = ctx.enter_context(tc.tile_pool(name="qk", bufs=3))
            v_pool = ctx.enter_context(tc.tile_pool(name="v", bufs=3))
            p_pool = ctx.enter_context(tc.tile_pool(name="p", bufs=3))
            s_pool = ctx.enter_context(tc.tile_pool(name="s", bufs=2))
            pt_pool = ctx.enter_context(tc.tile_pool(name="pt", bufs=3))
            ot_pool = ctx.enter_context(tc.tile_pool(name="ot", bufs=3))
            out_pool = ctx.enter_context(tc.tile_pool(name="out", bufs=3))

            ab_ps = ctx.enter_context(tc.tile_pool(name="abps", bufs=2, space="PSUM"))
            sc_ps = ctx.enter_context(tc.tile_pool(name="scps", bufs=3, space="PSUM"))
            tp_ps = ctx.enter_context(tc.tile_pool(name="tpps", bufs=1, space="PSUM"))
            av_ps = ctx.enter_context(tc.tile_pool(name="avps", bufs=1, space="PSUM"))
            pj_ps = ctx.enter_context(tc.tile_pool(name="pjps", bufs=1, space="PSUM"))

            # ---- constants ----
            wqT = consts.tile([128, 3, 384], bf16)
            wkT = consts.tile([128, 3, 384], bf16)
            wvT = consts.tile([128, 3, 384], bf16)
            pwT = consts.tile([128, 3, 384], f32r)
            nc.sync.dma_start(wqT[:], wq_d.ap())
            nc.sync.dma_start(wkT[:], wk_d.ap())
            nc.sync.dma_start(wvT[:], wv_d.ap())
            nc.sync.dma_start(pwT[:], pw_d.ap())
            expb0 = consts.tile([128, H, N], bf16)
            expb1 = consts.tile([68, H, N], bf16)
            nc.sync.dma_start(expb0[:], eb_d.ap()[0:128])
            nc.sync.dma_start(expb1[:], eb_d.ap()[128:N])
            ind_sb = consts.tile([H, 3, 128], bf16)
            nc.sync.dma_start(ind_sb[:], ind_d.ap())
            ident = consts.tile([128, 128], bf16)
            make_identity(nc, ident[:])
            ident32 = consts.tile([128, 128], f32)
            make_identity(nc, ident32[:])
            eps_t = consts.tile([128, 1], f32)
            nc.vector.memset(eps_t[:], EPS)
            # LN stats for all (window, chunk) pairs, computed in one
            # pre-phase so ACT only ever loads one function set (Ln/Exp once)
            nck = 2 * n_windows
            mv_all = consts.tile([128, nck, 2], f32)
            rstd_all = consts.tile([128, nck], f32)
            xb_all = consts.tile([128, nck, 384], bf16)
            nc.vector.memset(mv_all[:], 1.0)
            if has_bias:
                bqk = consts.tile([128, 6], f32)
                nc.sync.dma_start(bqk[:], bqk_d.ap())
                bv_b = consts.tile([128, 384], f32)
                pb_b = consts.tile([128, 384], f32)
                bvp_ap = bvp_d.ap()
                nc.sync.dma_start(
                    bv_b[:],
                    bass.AP(tensor=bvp_ap.tensor, offset=bvp_ap.offset,
                            ap=[[0, 128], [1, 384]]),
                )
                nc.sync.dma_start(
                    pb_b[:],
                    bass.AP(tensor=bvp_ap.tensor, offset=bvp_ap.offset + 384,
                            ap=[[0, 128], [1, 384]]),
                )

            lnv_all = consts.tile([128, 2 * n_windows], f32)
            GRP = 8

            def emit_stats_group(g0):
                g1 = min(g0 + GRP, n_windows)
                for w_ in range(g0, g1):
                    for ci_, (t0_, t1_) in enumerate(CH):
                        idx_ = 2 * w_ + ci_
                        tsz_ = t1_ - t0_
                        xt = x_pool.tile([128, 384], f32, tag="x", name="xt")
                        nc.sync.dma_start(xt[:tsz_], xa[w_, t0_:t1_, :])
                        stats = ln_pool.tile([128, 6], f32, tag="stats",
                                             name="stats")
                        nc.vector.bn_stats(stats[:tsz_], xt[:tsz_, :])
                        nc.vector.bn_aggr(mv_all[:tsz_, idx_, :], stats[:tsz_])
                        nc.gpsimd.tensor_copy(xb_all[:tsz_, idx_, :],
                                              xt[:tsz_, :])
                c0, c1 = 2 * g0, 2 * g1
                nc.scalar.activation(lnv_all[:, c0:c1], mv_all[:, c0:c1, 1],
                                     AF.Ln, bias=eps_t[:], scale=1.0)
                nc.scalar.activation(rstd_all[:, c0:c1], lnv_all[:, c0:c1],
                                     AF.Exp, bias=0.0, scale=-0.5)

            emit_stats_group(0)
            for w in range(n_windows):
                if w % GRP == 0 and w + GRP < n_windows:
                    emit_stats_group(w + GRP)
                # ---------- stage A: LN + transpose ----------
                xnT = xnt_pool.tile([128, 3, N], bf16)
                for ci, (t0, t1) in enumerate(CH):
                    idx = 2 * w + ci
                    tsz = t1 - t0
                    xn = xn_pool.tile([128, 384], bf16, tag="xn")
                    nc.vector.tensor_scalar(
                        out=xn[:tsz], in0=xb_all[:tsz, idx, :],
                        scalar1=mv_all[:tsz, idx, 0:1],
                        scalar2=rstd_all[:tsz, idx:idx + 1],
                        op0=ALU.subtract, op1=ALU.mult)
                    tp = ab_ps.tile([128, 3, 128], bf16, tag="ab")
                    for c in range(3):
                        nc.tensor.transpose(
                            tp[:, c, :tsz], xn[:tsz, c * 128:(c + 1) * 128],
                            ident[:tsz, :tsz])
                    nc.scalar.copy(xnT[:, :, t0:t1], tp[:, :, :tsz])

                # ---------- stage B: QKV ----------
                qT = qk_pool.tile([128, 3, N], bf16, tag="qT")
                kT = qk_pool.tile([128, 3, N], bf16, tag="kT")
                for dst, wT, bcol in ((qT, wqT, 0), (kT, wkT, 3)):
                    for fc in range(3):
                        ps = ab_ps.tile([128, 512], f32, tag="ab")
                        for dc in range(3):
                            nc.tensor.matmul(
                                ps[:, :N], wT[:, dc, fc * 128:(fc + 1) * 128],
                                xnT[:, dc, :], start=(dc == 0), stop=(dc == 2))
                        if has_bias:
                            nc.scalar.add(dst[:, fc, :], ps[:, :N],
                                          bqk[:, bcol + fc:bcol + fc + 1])
                        else:
                            nc.scalar.copy(dst[:, fc, :], ps[:, :N])
                # partition-shift heads to base 0: head h = 4*c + a lives at
                # qT[32a:32a+32, c, :]; SBUF->SBUF DMA can remap partitions
                qT32 = qk_pool.tile([32, H, N], bf16, tag="qT32")
                kT32 = qk_pool.tile([32, H, N], bf16, tag="kT32")
                for dst32, srcw in ((qT32, qT), (kT32, kT)):
                    for a in range(4):
                        nc.sync.dma_start(dst32[:, a::4, :],
                                          srcw[32 * a:32 * a + 32, :, :])
                if debug and w == 0:
                    nc.sync.dma_start(dbg["d_xnT"].ap(), xnT[:])
                    nc.sync.dma_start(dbg["d_qT"].ap(), qT[:])
                    nc.sync.dma_start(dbg["d_kT"].ap(), kT[:])
                vw = v_pool.tile([128, 2, 384], bf16)
                for ci, (t0, t1) in enumerate(CH):
                    tsz = t1 - t0
                    ps = ab_ps.tile([128, 512], f32, tag="ab")
                    for dc in range(3):
                        nc.tensor.matmul(
                            ps[:tsz, :384], xnT[:, dc, t0:t1], wvT[:, dc, :],
                            start=(dc == 0), stop=(dc == 2))
                    if has_bias:
                        nc.vector.tensor_add(vw[:tsz, ci, :], ps[:tsz, :384],
                                             bv_b[:tsz])
                    else:
                        nc.vector.tensor_copy(vw[:tsz, ci, :], ps[:tsz, :384])

                if debug and w == 0:
                    nc.sync.dma_start(dbg["d_vw"].ap()[:, 0, :], vw[:, 0, :])
                    nc.sync.dma_start(dbg["d_vw"].ap()[:68, 1, :], vw[:68, 1, :])
                # ---------- stage C: attention ----------
                pT = [pt_pool.tile([128, H, N], bf16, tag="pt0", name="pT0"),
                      pt_pool.tile([68, H, N], bf16, tag="pt1", name="pT1")]
                rrs = []
                for ni, (n0, n1) in enumerate(CH):
                    nsz = n1 - n0
                    P = p_pool.tile([128 if ni == 0 else 68, H, N], bf16,
                                    tag=f"p{ni}", name=f"P{ni}")
                    eb = expb0 if ni == 0 else expb1
                    for hp in range(6):  # passes of 2 heads, 1 PSUM bank each
                        sc = sc_ps.tile([128, 512], f32)
                        for i in range(2):
                            h = 2 * hp + i
                            nc.tensor.matmul(
                                sc[:nsz, i * N:i * N + N],
                                qT32[:, h, n0:n1], kT32[:, h, :],
                                start=True, stop=True)
                        nc.scalar.activation(
                            P[:nsz, 2 * hp:2 * hp + 2, :],
                            sc[:nsz, :2 * N], AF.Exp)
                    if stop in ("scores", "smm", "smm0", "smmtp"):
                        continue
                    s_t = s_pool.tile([128, H], f32, tag="s")
                    rr = s_pool.tile([128, H], f32, tag="rr")
                    if stop == "ttr":
                        for h in range(H):
                            nc.vector.tensor_tensor_reduce(
                                out=P[:nsz, h, :], in0=P[:nsz, h, :],
                                in1=eb[:nsz, h, :], scale=1.0, scalar=0.0,
                                op0=ALU.mult, op1=ALU.add,
                                accum_out=s_t[:nsz, h:h + 1])
                    else:
                        nc.vector.tensor_mul(P[:nsz], P[:nsz], eb[:nsz])
                        nc.vector.reduce_sum(s_t[:nsz], P[:nsz],
                                             mybir.AxisListType.X)
                    nc.vector.reciprocal(rr[:nsz], s_t[:nsz])
                    rrs.append(rr)
                    if debug and w == 0 and ni == 0:
                        nc.sync.dma_start(dbg["d_P0"].ap(), P[:])
                    if stop == "softmax":
                        continue
                    for mi, (m0, m1) in enumerate(CH):
                        msz = m1 - m0
                        for hh in range(2):  # halves of 6 heads
                            tp = tp_ps.tile([128, 6, 128], bf16)
                            for i in range(6):
                                h = 6 * hh + i
                                nc.tensor.transpose(
                                    tp[:msz, i, :nsz], P[:nsz, h, m0:m1],
                                    ident[:nsz, :nsz])
                            cp_out = pT[mi][:msz, 6 * hh:6 * hh + 6, n0:n1]
                            if hh == 0 and mi == 0:
                                nc.vector.tensor_copy(cp_out, tp[:msz, :, :nsz])
                            else:
                                nc.scalar.copy(cp_out, tp[:msz, :, :nsz])

                if stop in ("scores", "smm", "smm0", "smmtp", "softmax", "sfm1", "sfm2", "sfm3", "transp"):
                    zo = out_pool.tile([128, 384], f32, tag="o")
                    nc.vector.memset(zo[:], 0.0)
                    for (t0, t1) in CH:
                        nc.sync.dma_start(outa[w, t0:t1, :], zo[:t1 - t0])
                    continue
                if debug and w == 0:
                    nc.sync.dma_start(dbg["d_pT0"].ap(), pT[0][:])
                rrT = sc_ps.tile([12, 512], f32, tag="sc", name="rrT")
                for ni, (n0, n1) in enumerate(CH):
                    nsz = n1 - n0
                    nc.tensor.transpose(rrT[:, n0:n1], rrs[ni][:nsz, :],
                                        ident32[:nsz, :nsz])
                rrT_sb = s_pool.tile([12, N], bf16, tag="rrTs", name="rrT_sb")
                nc.vector.tensor_copy(rrT_sb[:], rrT[:, :N])
                rrx_sb = ot_pool.tile([128, 3, N], f32, tag="rrx", name="rrx_sb")
                for qd in range(3):
                    rrx = sc_ps.tile([128, 512], f32, tag="sc", name="rrx")
                    nc.tensor.matmul(rrx[:, :N], ind_sb[:, qd, :], rrT_sb[:],
                                     start=True, stop=True)
                    nc.scalar.copy(rrx_sb[:, qd, :], rrx[:, :N])

                oT = ot_pool.tile([128, 3, N], f32r)
                for qd in range(3):
                    av = av_ps.tile([128, 512], f32)
                    for i in range(4):
                        h = 4 * qd + i
                        for mi, (m0, m1) in enumerate(CH):
                            msz = m1 - m0
                            nc.tensor.matmul(
                                av[32 * i:32 * i + 32, :N],
                                vw[:msz, mi, 32 * h:32 * h + 32],
                                pT[mi][:msz, h, :],
                                start=(mi == 0), stop=(mi == 1),
                                tile_position=(0, 32 * i))
                    nc.vector.tensor_mul(oT[:, qd, :], av[:, :N],
                                         rrx_sb[:, qd, :])

                if stop == "av":
                    zo = out_pool.tile([128, 384], f32, tag="o")
                    nc.vector.memset(zo[:], 0.0)
                    for (t0, t1) in CH:
                        nc.sync.dma_start(outa[w, t0:t1, :], zo[:t1 - t0])
                    continue
                if debug and w == 0:
                    nc.sync.dma_start(dbg["d_oT"].ap(), oT[:])
                for ci, (t0, t1) in enumerate(CH):
                    tsz = t1 - t0
                    pj = pj_ps.tile([128, 512], f32)
                    for qd in range(3):
                        nc.tensor.matmul(
                            pj[:tsz, :384],
                            oT[:, qd, t0:t1],
                            pwT[:, qd, :],
                            start=(qd == 0), stop=(qd == 2))
                    ot = out_pool.tile([128, 384], f32, tag="o")
                    if has_bias:
                        nc.vector.tensor_add(ot[:tsz], pj[:tsz, :384],
                                             pb_b[:tsz])
                    else:
                        nc.vector.tensor_copy(ot[:tsz], pj[:tsz, :384])
                    nc.sync.dma_start(outa[w, t0:t1, :], ot[:tsz])

    nc.compile()
    return nc


def _prep(norm_w, norm_b, qkv_w, qkv_b, attention_biases, proj_w, proj_b,
          bias_idxs):
    """Host-side weight prep: fold LN gamma/beta + q-scale into weights,
    reorder rows q/k/v-grouped, transpose to [d, f] SBUF layouts."""
    scale = np.float32(KD ** -0.5)
    qkv3 = qkv_w.reshape(H, 2 * KD + VD, DIM)
    b3 = qkv_b.reshape(H, 2 * KD + VD)
    wq = qkv3[:, :KD, :].reshape(H * KD, DIM) * scale
    wk = qkv3[:, KD:2 * KD, :].reshape(H * KD, DIM)
    wv = qkv3[:, 2 * KD:, :].reshape(H * VD, DIM)
    bq = b3[:, :KD].reshape(-1) * scale
    bk = b3[:, KD:2 * KD].reshape(-1)
    bv = b3[:, 2 * KD:].reshape(-1)
    # fold LN: qkv = W(g*z + b) = (W*g) z + W b
    wq_e = wq * norm_w[None, :]
    wk_e = wk * norm_w[None, :]
    wv_e = wv * norm_w[None, :]
    bq_e = bq + wq @ norm_b
    bk_e = bk + wk @ norm_b
    bv_e = bv + wv @ norm_b

    def chunk_T(m, dt):  # [f, d] -> [128, 3, f-dim] (d-major chunks)
        return np.ascontiguousarray(
            m.T.reshape(3, 128, m.shape[0]).transpose(1, 0, 2)).astype(dt)

    wqT = chunk_T(wq_e, BF16)
    wkT = chunk_T(wk_e, BF16)
    wvT = chunk_T(wv_e, BF16)
    # proj contraction is over f: pwT[p, qd, d] = proj_w[d, 128*qd+p]
    pwT = chunk_T(proj_w, np.float32)

    # indicator for PE expansion of 1/s: ind[h, qd, p] = (h == 4*qd + p//32)
    ind = np.zeros((H, 3, 128), np.float32)
    for qd in range(3):
        for i in range(4):
            ind[4 * qd + i, qd, 32 * i:32 * i + 32] = 1.0
    ind = ind.astype(BF16)

    bias_g = attention_biases[:, bias_idxs]  # [H, N, N]
    expb = np.exp(bias_g).transpose(1, 0, 2).astype(BF16)  # [n, h, m]
    expb = np.ascontiguousarray(expb)

    has_bias = bool(np.any(bq_e) or np.any(bk_e) or np.any(bv_e)
                    or np.any(proj_b))
    bqk = np.concatenate([bq_e.reshape(3, 128), bk_e.reshape(3, 128)], 0)
    bqk = np.ascontiguousarray(bqk.T).astype(np.float32)
    bvp = np.stack([bv_e, proj_b]).astype(np.float32)
    return wqT, wkT, wvT, pwT, expb, bqk, bvp, has_bias, ind


def kernel(x, norm_w, norm_b, qkv_w, qkv_b, attention_biases, proj_w, proj_b,
           bias_idxs):
    from concourse.bass_utils import run_bass_kernel_spmd

    x = np.asarray(x, np.float32)
    wqT, wkT, wvT, pwT, expb, bqk, bvp, has_bias, ind = _prep(
        np.asarray(norm_w, np.float32), np.asarray(norm_b, np.float32),
        np.asarray(qkv_w, np.float32), np.asarray(qkv_b, np.float32),
        np.asarray(attention_biases, np.float32),
        np.asarray(proj_w, np.float32), np.asarray(proj_b, np.float32),
        np.asarray(bias_idxs))

    key = (WPC, has_bias)
    if key not in _cache:
        _cache[key] = _build(WPC, has_bias)
    nc = _cache[key]

    base = {"wqT": wqT, "wkT": wkT, "wvT": wvT, "pwT": pwT, "expb": expb,
            "ind": ind}
    if has_bias:
        base["bqk"] = bqk
        base["bvp"] = bvp
    in_maps = []
    for i in range(NCORES):
        m = dict(base)
        m["x"] = np.ascontiguousarray(x[i * WPC:(i + 1) * WPC])
        in_maps.append(m)

    res = run_bass_kernel_spmd(nc, in_maps, core_ids=list(range(NCORES)))
    global last_results
    last_results = res
    out = np.concatenate([res.results[i]["out"] for i in range(NCORES)], 0)
    return out.astype(np.float32)


# revision 53
# speedup vs baseline: 1.2367x; 1.2367x over previous
"""Windowed attention block (LeViT/Swin-style) on Trainium2, 8 NeuronCores.

LayerNorm -> QKV -> per-head biased softmax attention -> projection for
B=256 windows, N=196 tokens, DIM=384, 12 heads of head-dim 32.

Sharding: data-parallel over windows (32 windows/core, 8 cores); params
replicated. Per core the Bass/Tile kernel processes windows one at a time:

  stage A: x tile [t<=128, 384] -> LN stats (bn_stats) -> xn bf16
           -> PE-transpose -> xnT [384, 196] (feature-major)
  stage B: QKV. q/k feature-major ([384 f, 196 t], head-grouped rows,
           scale folded into wq), v token-major ([196 t, 384 f]).
  stage C: per head: scores = qT.T @ kT -> PSUM [n, m]; softmax along
           free dim (exp on ACT, x exp(bias), rowsum, reciprocal,
           normalize with a zero-stride broadcast multiply); PE-transpose
           probs -> probsT [m, n]; AV: out = v.T @ probsT -> oT [f, n];
           proj (fp32r) -> [t, 384] -> DMA out.

All matmuls bf16 except the projection (fp32r). LN gamma/beta are folded
into the QKV weights on the host; softmax max-subtraction is skipped
(scores are O(1) by construction: weights ~N(0, 0.02^2)).
"""

import sys

sys.path.insert(0, "/opt/trn_rl_repo")

import numpy as np
import ml_dtypes

B, N, DIM = 256, 196, 384
H, KD, VD = 12, 32, 32
EPS = 1e-5
NCORES = 8
WPC = B // NCORES  # windows per core

BF16 = ml_dtypes.bfloat16

_cache = {}
last_results = None


STOP_AFTER = None  # bisect hook: "A","B","scores","softmax","transp","av",None


def _build(n_windows, has_bias, debug=False):
    stop = STOP_AFTER
    import concourse.bass as bass
    import concourse.bacc as bacc
    import concourse.tile as tile
    from concourse import mybir
    from concourse.masks import make_identity

    f32 = mybir.dt.float32
    f32r = mybir.dt.float32r
    bf16 = mybir.dt.bfloat16
    AF = mybir.ActivationFunctionType
    ALU = mybir.AluOpType

    nc = bacc.Bacc("TRN2", target_bir_lowering=False, debug=False)

    x_d = nc.dram_tensor("x", [n_windows, N, DIM], f32, kind="ExternalInput")
    wq_d = nc.dram_tensor("wqT", [128, 3, 384], bf16, kind="ExternalInput")
    wk_d = nc.dram_tensor("wkT", [128, 3, 384], bf16, kind="ExternalInput")
    wv_d = nc.dram_tensor("wvT", [128, 3, 384], bf16, kind="ExternalInput")
    pw_d = nc.dram_tensor("pwT", [128, 3, 384], f32r, kind="ExternalInput")
    eb_d = nc.dram_tensor("expb", [N, H, N], bf16, kind="ExternalInput")
    if has_bias:
        bqk_d = nc.dram_tensor("bqk", [128, 6], f32, kind="ExternalInput")
        bvp_d = nc.dram_tensor("bvp", [2, 384], f32, kind="ExternalInput")
    out_d = nc.dram_tensor("out", [n_windows, N, DIM], f32, kind="ExternalOutput")
    if debug:
        dbg = {
            "d_xnT": nc.dram_tensor("d_xnT", [128, 3, N], bf16, kind="ExternalOutput"),
            "d_qT": nc.dram_tensor("d_qT", [128, 3, N], bf16, kind="ExternalOutput"),
            "d_kT": nc.dram_tensor("d_kT", [128, 3, N], bf16, kind="ExternalOutput"),
            "d_vw": nc.dram_tensor("d_vw", [128, 2, 384], bf16, kind="ExternalOutput"),
            "d_P0": nc.dram_tensor("d_P0", [128, H, N], bf16, kind="ExternalOutput"),
            "d_pT0": nc.dram_tensor("d_pT0", [128, H, N], bf16, kind="ExternalOutput"),
            "d_oT": nc.dram_tensor("d_oT", [128, 3, N], f32r, kind="ExternalOutput"),
        }

    xa, outa = x_d.ap(), out_d.ap()

    # token chunks of a window: [0,128) and [128,196)
    CH = [(0, 128), (128, N)]

    with tile.TileContext(nc) as tc:
        import contextlib

        ctx = contextlib.ExitStack()
        with ctx:
            consts = ctx.enter_context(tc.tile_pool(name="consts", bufs=1))
            x_pool = ctx.enter_context(tc.tile_pool(name="x", bufs=3))
            ln_pool = ctx.enter_context(tc.tile_pool(name="ln", bufs=4))
            xn_pool = ctx.enter_context(tc.tile_pool(name="xn", bufs=3))
            xnt_pool = ctx.enter_context(tc.tile_pool(name="xnt", bufs=3))
            qk_pool = ctx.enter_context(tc.tile_pool(name="qk", bufs=3))
            v_pool = ctx.enter_context(tc.tile_pool(name="v", bufs=3))
            p_pool = ctx.enter_context(tc.tile_pool(name="p", bufs=3))
            s_pool = ctx.enter_context(tc.tile_pool(name="s", bufs=2))
            pt_pool = ctx.enter_context(tc.tile_pool(name="pt", bufs=3))
            ot_pool = ctx.enter_context(tc.tile_pool(name="ot", bufs=3))
            out_pool = ctx.enter_context(tc.tile_pool(name="out", bufs=3))

            ab_ps = ctx.enter_context(tc.tile_pool(name="abps", bufs=2, space="PSUM"))
            sc_ps = ctx.enter_context(tc.tile_pool(name="scps", bufs=3, space="PSUM"))
            tp_ps = ctx.enter_context(tc.tile_pool(name="tpps", bufs=1, space="PSUM"))
            av_ps = ctx.enter_context(tc.tile_pool(name="avps", bufs=1, space="PSUM"))
            pj_ps = ctx.enter_context(tc.tile_pool(name="pjps", bufs=1, space="PSUM"))

            # ---- constants ----
            wqT = consts.tile([128, 3, 384], bf16)
            wkT = consts.tile([128, 3, 384], bf16)
            wvT = consts.tile([128, 3, 384], bf16)
            pwT = consts.tile([128, 3, 384], f32r)
            nc.sync.dma_start(wqT[:], wq_d.ap())
            nc.sync.dma_start(wkT[:], wk_d.ap())
            nc.sync.dma_start(wvT[:], wv_d.ap())
            nc.sync.dma_start(pwT[:], pw_d.ap())
            expb0 = consts.tile([128, H, N], bf16)
            expb1 = consts.tile([68, H, N], bf16)
            nc.sync.dma_start(expb0[:], eb_d.ap()[0:128])
            nc.sync.dma_start(expb1[:], eb_d.ap()[128:N])
            ident = consts.tile([128, 128], bf16)
            make_identity(nc, ident[:])
            eps_t = consts.tile([128, 1], f32)
            nc.vector.memset(eps_t[:], EPS)
            # LN stats for all (window, chunk) pairs, computed in one
            # pre-phase so ACT only ever loads one function set (Ln/Exp once)
            nck = 2 * n_windows
            mv_all = consts.tile([128, nck, 2], f32)
            rstd_all = consts.tile([128, nck], f32)
            xb_all = consts.tile([128, nck, 384], bf16)
            nc.vector.memset(mv_all[:], 1.0)
            if has_bias:
                bqk = consts.tile([128, 6], f32)
                nc.sync.dma_start(bqk[:], bqk_d.ap())
                bv_b = consts.tile([128, 384], f32)
                pb_b = consts.tile([128, 384], f32)
                bvp_ap = bvp_d.ap()
                nc.sync.dma_start(
                    bv_b[:],
                    bass.AP(tensor=bvp_ap.tensor, offset=bvp_ap.offset,
                            ap=[[0, 128], [1, 384]]),
                )
                nc.sync.dma_start(
                    pb_b[:],
                    bass.AP(tensor=bvp_ap.tensor, offset=bvp_ap.offset + 384,
                            ap=[[0, 128], [1, 384]]),
                )

            lnv_all = consts.tile([128, 2 * n_windows], f32)
            GRP = 8

            def emit_stats_group(g0):
                g1 = min(g0 + GRP, n_windows)
                for w_ in range(g0, g1):
                    for ci_, (t0_, t1_) in enumerate(CH):
                        idx_ = 2 * w_ + ci_
                        tsz_ = t1_ - t0_
                        xt = x_pool.tile([128, 384], f32, tag="x", name="xt")
                        nc.sync.dma_start(xt[:tsz_], xa[w_, t0_:t1_, :])
                        stats = ln_pool.tile([128, 6], f32, tag="stats",
                                             name="stats")
                        nc.vector.bn_stats(stats[:tsz_], xt[:tsz_, :])
                        nc.vector.bn_aggr(mv_all[:tsz_, idx_, :], stats[:tsz_])
                        nc.gpsimd.tensor_copy(xb_all[:tsz_, idx_, :],
                                              xt[:tsz_, :])
                c0, c1 = 2 * g0, 2 * g1
                nc.scalar.activation(lnv_all[:, c0:c1], mv_all[:, c0:c1, 1],
                                     AF.Ln, bias=eps_t[:], scale=1.0)
                nc.scalar.activation(rstd_all[:, c0:c1], lnv_all[:, c0:c1],
                                     AF.Exp, bias=0.0, scale=-0.5)

            emit_stats_group(0)
            for w in range(n_windows):
                if w % GRP == 0 and w + GRP < n_windows:
                    emit_stats_group(w + GRP)
                # ---------- stage A: LN + transpose ----------
                xnT = xnt_pool.tile([128, 3, N], bf16)
                for ci, (t0, t1) in enumerate(CH):
                    idx = 2 * w + ci
                    tsz = t1 - t0
                    xn = xn_pool.tile([128, 384], bf16, tag="xn")
                    nc.vector.tensor_scalar(
                        out=xn[:tsz], in0=xb_all[:tsz, idx, :],
                        scalar1=mv_all[:tsz, idx, 0:1],
                        scalar2=rstd_all[:tsz, idx:idx + 1],
                        op0=ALU.subtract, op1=ALU.mult)
                    tp = ab_ps.tile([128, 3, 128], bf16, tag="ab")
                    for c in range(3):
                        nc.tensor.transpose(
                            tp[:, c, :tsz], xn[:tsz, c * 128:(c + 1) * 128],
                            ident[:tsz, :tsz])
                    nc.scalar.copy(xnT[:, :, t0:t1], tp[:, :, :tsz])

                # ---------- stage B: QKV ----------
                qT = qk_pool.tile([128, 3, N], bf16, tag="qT")
                kT = qk_pool.tile([128, 3, N], bf16, tag="kT")
                for dst, wT, bcol in ((qT, wqT, 0), (kT, wkT, 3)):
                    for fc in range(3):
                        ps = ab_ps.tile([128, 512], f32, tag="ab")
                        for dc in range(3):
                            nc.tensor.matmul(
                                ps[:, :N], wT[:, dc, fc * 128:(fc + 1) * 128],
                                xnT[:, dc, :], start=(dc == 0), stop=(dc == 2))
                        if has_bias:
                            nc.scalar.add(dst[:, fc, :], ps[:, :N],
                                          bqk[:, bcol + fc:bcol + fc + 1])
                        else:
                            nc.scalar.copy(dst[:, fc, :], ps[:, :N])
                # partition-shift heads to base 0: head h = 4*c + a lives at
                # qT[32a:32a+32, c, :]; SBUF->SBUF DMA can remap partitions
                qT32 = qk_pool.tile([32, H, N], bf16, tag="qT32")
                kT32 = qk_pool.tile([32, H, N], bf16, tag="kT32")
                for dst32, srcw in ((qT32, qT), (kT32, kT)):
                    for a in range(4):
                        nc.sync.dma_start(dst32[:, a::4, :],
                                          srcw[32 * a:32 * a + 32, :, :])
                if debug and w == 0:
                    nc.sync.dma_start(dbg["d_xnT"].ap(), xnT[:])
                    nc.sync.dma_start(dbg["d_qT"].ap(), qT[:])
                    nc.sync.dma_start(dbg["d_kT"].ap(), kT[:])
                vw = v_pool.tile([128, 2, 384], bf16)
                for ci, (t0, t1) in enumerate(CH):
                    tsz = t1 - t0
                    ps = ab_ps.tile([128, 512], f32, tag="ab")
                    for dc in range(3):
                        nc.tensor.matmul(
                            ps[:tsz, :384], xnT[:, dc, t0:t1], wvT[:, dc, :],
                            start=(dc == 0), stop=(dc == 2))
                    if has_bias:
                        nc.vector.tensor_add(vw[:tsz, ci, :], ps[:tsz, :384],
                                             bv_b[:tsz])
                    else:
                        nc.vector.tensor_copy(vw[:tsz, ci, :], ps[:tsz, :384])

                if debug and w == 0:
                    nc.sync.dma_start(dbg["d_vw"].ap()[:, 0, :], vw[:, 0, :])
                    nc.sync.dma_start(dbg["d_vw"].ap()[:68, 1, :], vw[:68, 1, :])
                # ---------- stage C: attention ----------
                pT = [pt_pool.tile([128, H, N], bf16, tag="pt0", name="pT0"),
                      pt_pool.tile([68, H, N], bf16, tag="pt1", name="pT1")]
                for ni, (n0, n1) in enumerate(CH):
                    nsz = n1 - n0
                    P = p_pool.tile([128 if ni == 0 else 68, H, N], bf16,
                                    tag=f"p{ni}", name=f"P{ni}")
                    eb = expb0 if ni == 0 else expb1
                    for hp in range(6):  # passes of 2 heads, 1 PSUM bank each
                        sc = sc_ps.tile([128, 512], f32)
                        for i in range(2):
                            h = 2 * hp + i
                            nc.tensor.matmul(
                                sc[:nsz, i * N:i * N + N],
                                qT32[:, h, n0:n1], kT32[:, h, :],
                                start=True, stop=True)
                        nc.scalar.activation(
                            P[:nsz, 2 * hp:2 * hp + 2, :],
                            sc[:nsz, :2 * N], AF.Exp)
                    if stop in ("scores", "smm", "smm0", "smmtp"):
                        continue
                    s_t = s_pool.tile([128, H], f32, tag="s")
                    rr = s_pool.tile([128, H], f32, tag="rr")
                    if stop == "ttr":
                        for h in range(H):
                            nc.vector.tensor_tensor_reduce(
                                out=P[:nsz, h, :], in0=P[:nsz, h, :],
                                in1=eb[:nsz, h, :], scale=1.0, scalar=0.0,
                                op0=ALU.mult, op1=ALU.add,
                                accum_out=s_t[:nsz, h:h + 1])
                    else:
                        nc.vector.tensor_mul(P[:nsz], P[:nsz], eb[:nsz])
                        nc.vector.reduce_sum(s_t[:nsz], P[:nsz],
                                             mybir.AxisListType.X)
                    nc.vector.reciprocal(rr[:nsz], s_t[:nsz])
                    nc.vector.tensor_mul(P[:nsz], P[:nsz],
                                         rr[:nsz].to_broadcast((nsz, H, N)))
                    if debug and w == 0 and ni == 0:
                        nc.sync.dma_start(dbg["d_P0"].ap(), P[:])
                    if stop == "softmax":
                        continue
                    for mi, (m0, m1) in enumerate(CH):
                        msz = m1 - m0
                        for hh in range(2):  # halves of 6 heads
                            tp = tp_ps.tile([128, 6, 128], bf16)
                            for i in range(6):
                                h = 6 * hh + i
                                nc.tensor.transpose(
                                    tp[:msz, i, :nsz], P[:nsz, h, m0:m1],
                                    ident[:nsz, :nsz])
                            cp_out = pT[mi][:msz, 6 * hh:6 * hh + 6, n0:n1]
                            if hh == 0 and mi == 0:
                                nc.vector.tensor_copy(cp_out, tp[:msz, :, :nsz])
                            else:
                                nc.scalar.copy(cp_out, tp[:msz, :, :nsz])

                if stop in ("scores", "smm", "smm0", "smmtp", "softmax", "sfm1", "sfm2", "sfm3", "transp"):
                    zo = out_pool.tile([128, 384], f32, tag="o")
                    nc.vector.memset(zo[:], 0.0)
                    for (t0, t1) in CH:
                        nc.sync.dma_start(outa[w, t0:t1, :], zo[:t1 - t0])
                    continue
                if debug and w == 0:
                    nc.sync.dma_start(dbg["d_pT0"].ap(), pT[0][:])
                oT = ot_pool.tile([128, 3, N], f32r)
                for qd in range(3):
                    av = av_ps.tile([128, 512], f32)
                    for i in range(4):
                        h = 4 * qd + i
                        for mi, (m0, m1) in enumerate(CH):
                            msz = m1 - m0
                            nc.tensor.matmul(
                                av[32 * i:32 * i + 32, :N],
                                vw[:msz, mi, 32 * h:32 * h + 32],
                                pT[mi][:msz, h, :],
                                start=(mi == 0), stop=(mi == 1),
                                tile_position=(0, 32 * i))
                    nc.vector.tensor_copy(oT[:, qd, :], av[:, :N])

                if stop == "av":
                    zo = out_pool.tile([128, 384], f32, tag="o")
                    nc.vector.memset(zo[:], 0.0)
                    for (t0, t1) in CH:
                        nc.sync.dma_start(outa[w, t0:t1, :], zo[:t1 - t0])
                    continue
                if debug and w == 0:
                    nc.sync.dma_start(dbg["d_oT"].ap(), oT[:])
                for ci, (t0, t1) in enumerate(CH):
                    tsz = t1 - t0
                    pj = pj_ps.tile([128, 512], f32)
                    for qd in range(3):
                        nc.tensor.matmul(
                            pj[:tsz, :384],
                            oT[:, qd, t0:t1],
                            pwT[:, qd, :],
                            start=(qd == 0), stop=(qd == 2))
                    ot = out_pool.tile([128, 384], f32, tag="o")
                    if has_bias:
                        nc.vector.tensor_add(ot[:tsz], pj[:tsz, :384],
                                             pb_b[:tsz])
                    else:
                        nc.vector.tensor_copy(ot[:tsz], pj[:tsz, :384])
                    nc.sync.dma_start(outa[w, t0:t1, :], ot[:tsz])

    nc.compile()
    return nc


def _prep(norm_w, norm_b, qkv_w, qkv_b, attention_biases, proj_w, proj_b,
          bias_idxs):
    """Host-side weight prep: fold LN gamma/beta + q-scale into weights,
    reorder rows q/k/v-grouped, transpose to [d, f] SBUF layouts."""
    scale = np.float32(KD ** -0.5)
    qkv3 = qkv_w.reshape(H, 2 * KD + VD, DIM)
    b3 = qkv_b.reshape(H, 2 * KD + VD)
    wq = qkv3[:, :KD, :].reshape(H * KD, DIM) * scale
    wk = qkv3[:, KD:2 * KD, :].reshape(H * KD, DIM)
    wv = qkv3[:, 2 * KD:, :].reshape(H * VD, DIM)
    bq = b3[:, :KD].reshape(-1) * scale
    bk = b3[:, KD:2 * KD].reshape(-1)
    bv = b3[:, 2 * KD:].reshape(-1)
    # fold LN: qkv = W(g*z + b) = (W*g) z + W b
    wq_e = wq * norm_w[None, :]
    wk_e = wk * norm_w[None, :]
    wv_e = wv * norm_w[None, :]
    bq_e = bq + wq @ norm_b
    bk_e = bk + wk @ norm_b
    bv_e = bv + wv @ norm_b

    def chunk_T(m, dt):  # [f, d] -> [128, 3, f-dim] (d-major chunks)
        return np.ascontiguousarray(
            m.T.reshape(3, 128, m.shape[0]).transpose(1, 0, 2)).astype(dt)

    wqT = chunk_T(wq_e, BF16)
    wkT = chunk_T(wk_e, BF16)
    wvT = chunk_T(wv_e, BF16)
    # proj contraction is over f: pwT[p, qd, d] = proj_w[d, 128*qd+p]
    pwT = chunk_T(proj_w, np.float32)

    bias_g = attention_biases[:, bias_idxs]  # [H, N, N]
    expb = np.exp(bias_g).transpose(1, 0, 2).astype(BF16)  # [n, h, m]
    expb = np.ascontiguousarray(expb)

    has_bias = bool(np.any(bq_e) or np.any(bk_e) or np.any(bv_e)
                    or np.any(proj_b))
    bqk = np.concatenate([bq_e.reshape(3, 128), bk_e.reshape(3, 128)], 0)
    bqk = np.ascontiguousarray(bqk.T).astype(np.float32)
    bvp = np.stack([bv_e, proj_b]).astype(np.float32)
    return wqT, wkT, wvT, pwT, expb, bqk, bvp, has_bias


def kernel(x, norm_w, norm_b, qkv_w, qkv_b, attention_biases, proj_w, proj_b,
           bias_idxs):
    from concourse.bass_utils import run_bass_kernel_spmd

    x = np.asarray(x, np.float32)
    wqT, wkT, wvT, pwT, expb, bqk, bvp, has_bias = _prep(
        np.asarray(norm_w, np.float32), np.asarray(norm_b, np.float32),
        np.asarray(qkv_w, np.float32), np.asarray(qkv_b, np.float32),
        np.asarray(attention_biases, np.float32),
        np.asarray(proj_w, np.float32), np.asarray(proj_b, np.float32),
        np.asarray(bias_idxs))

    key = (WPC, has_bias)
    if key not in _cache:
        _cache[key] = _build(WPC, has_bias)
    nc = _cache[key]

    base = {"wqT": wqT, "wkT": wkT, "wvT": wvT, "pwT": pwT, "expb": expb}
    if has_bias:
        base["bqk"] = bqk
        base["bvp"] = bvp
    in_maps = []
    for i in range(NCORES):
        m = dict(base)
        m["x"] = np.ascontiguousarray(x[i * WPC:(i + 1) * WPC])
        in_maps.append(m)

    res = run_bass_kernel_spmd(nc, in_maps, core_ids=list(range(NCORES)))
    global last_results
    last_results = res
    out = np.concatenate([res.results[i]["out"] for i in range(NCORES)], 0)
    return out.astype(np.float32)
